# revision 30
# baseline (speedup 1.0000x reference)
"""Trainium2 Bass kernel for a single causal attention head (prefill).

Problem shapes (hardcoded): x [4, 4096, 2048], W_Q/W_K/W_V [2048, 128].
reference: Q = rope(x@W_Q), K = x@W_K, V = rope(x@W_V),
           out = softmax(causal(Q K^T / sqrt(128))) @ V.

Sharding: 8 cores = (batch b, stripe h in {0,1}).  Core (b,h) computes the
output rows of batch b belonging to the interleaved 128-row blocks
g = 2j + h (j = 0..15) — balancing the causal triangle between the two
cores of a batch.  Each core computes the full K/V of its batch locally
(no collectives).

On-chip layout: everything transposed.  The host passes x^T per batch, so
projections contract over E with W e-tiles stationary and x^T moving,
producing Q^T/K^T/V^T [Dh=128 part, tok free] directly.  RoPE pairs are
made partition-contiguous by permuting W columns (even dims first) on the
host.  Scores are computed transposed (S^T[k, q]); softmax skips the max
subtraction (scores are gaussian-bounded, exp stays in fp32 range); exp
runs on ACT; P^T @ V and the softmax denominator (ones-matmul) accumulate
in a single PSUM bank per 256-row q-pair; the output is produced
transposed and rearranged on the host.

SPMD trick for the h stripe: both cores run one program that always
gathers q columns at window offsets {0, 256}.  For h=1 cores the host
rotates every 512-token window of x^T (and the rope tables) left by 128
columns, so those offsets select the h=1 blocks; K/V column order inside a
window changes with it, which only matters for the causal mask of the
diagonal k-tile — and the host builds that mask for the rotated order.
"""

import os
import sys

for _p in (
    "/root/.axon_site",
    "/root/.axon_site/_ro/trn_rl_repo",
    "/root/.axon_site/_ro/pypackages",
    "/opt/trn_rl_repo",
):
    if os.path.isdir(_p) and _p not in sys.path:
        sys.path.append(_p)

import numpy as np
import ml_dtypes

import concourse.mybir as mybir
import concourse.tile as tile
from concourse import bacc
from concourse.bass_utils import run_bass_kernel_spmd
from concourse.masks import make_identity

B, S, E, DH = 4, 4096, 2048, 128
NE = E // 128            # 16 e-tiles
NW = 8                   # windows of 512 tokens
WTOK = S // NW           # 512
NPAIR = 8                # q pairs per core (each 256 q rows)
NKSUB = S // 128         # 32 global k sub-blocks of 128
SCALE = 1.0 / float(np.sqrt(np.float64(DH)))
F32, F32R, BF16 = mybir.dt.float32, mybir.dt.float32r, mybir.dt.bfloat16
XDT = BF16  # dtype of x^T and W inputs (BF16 halves DMA; F32R for accuracy)

_CACHE = {}


def _build(debug=False):
    nc = bacc.Bacc()

    xt = nc.dram_tensor("xt", [NW, 128, NE, WTOK], XDT, kind="ExternalInput")
    wq = nc.dram_tensor("wq", [128, NE, DH], XDT, kind="ExternalInput")
    wk = nc.dram_tensor("wk", [128, NE, DH], XDT, kind="ExternalInput")
    wv = nc.dram_tensor("wv", [128, NE, DH], XDT, kind="ExternalInput")
    cosv = nc.dram_tensor("cosv", [64, S], F32, kind="ExternalInput")
    sinv = nc.dram_tensor("sinv", [64, S], F32, kind="ExternalInput")
    maskt = nc.dram_tensor("maskt", [128, 4, 256], BF16, kind="ExternalInput")
    outt = nc.dram_tensor("outt", [128, 16, 128], F32, kind="ExternalOutput")
    if debug:
        kt_dbg = nc.dram_tensor("kt_dbg", [128, S], F32R, kind="ExternalOutput")
        qt_dbg = nc.dram_tensor("qt_dbg", [128, 2048], F32R, kind="ExternalOutput")
        vn_dbg = nc.dram_tensor("vn_dbg", [128, NKSUB, 132], BF16, kind="ExternalOutput")
        acc_dbg = nc.dram_tensor("acc_dbg", [8, 2, 128, 129], F32, kind="ExternalOutput")

    with tile.TileContext(nc) as tc:
        with (
            tc.tile_pool(name="consts", bufs=1) as consts,
            tc.tile_pool(name="big", bufs=1) as big,
            tc.tile_pool(name="xs", bufs=3) as xs,
            tc.tile_pool(name="work", bufs=3) as work,
            tc.tile_pool(name="rope", bufs=1) as rope,
            tc.tile_pool(name="pt", bufs=6) as ptp,
            tc.tile_pool(name="fin", bufs=2) as fin,
            tc.tile_pool(name="ppsum", bufs=2, space="PSUM") as ppsum,
            tc.tile_pool(name="spsum", bufs=2, space="PSUM") as spsum,
            tc.tile_pool(name="apsum", bufs=1, space="PSUM") as apsum,
        ):
            w_sb = {}
            for name, dram in (("wk", wk), ("wv", wv), ("wq", wq)):
                t = consts.tile([128, NE, DH], XDT, tag=name)
                if name == "wk":
                    nc.scalar.dma_start(out=t[:, 0:2, :], in_=dram[:, 0:2, :])
                    nc.scalar.dma_start(out=t[:, 2:NE, :], in_=dram[:, 2:NE, :])
                else:
                    nc.scalar.dma_start(out=t, in_=dram[:, :, :])
                w_sb[name] = t
            mask_sb = consts.tile([128, 4, 256], BF16, tag="maskt")
            nc.scalar.dma_start(out=mask_sb, in_=maskt[:, :, :])

            ident = consts.tile([128, 128], BF16, tag="ident")
            make_identity(nc, ident)

            kt_sb = big.tile([128, S], F32R, tag="kt")
            qt_sb = big.tile([128, 2048], F32R, tag="qt")
            vn_sb = big.tile([128, NKSUB, 132], BF16, tag="vn")
            nc.vector.memset(vn_sb[:, :, 128:129], 1.0)
            out_sb = big.tile([128, 16, 128], F32, tag="outt")

            # ---- projections + rope, one 512-token window at a time ----
            for w in range(NW):
                xc = xs.tile([128, NE, WTOK], XDT, tag="xc")
                xin = xt[w, :, :, :]
                if w == 0:
                    for q8 in range(8):
                        nc.sync.dma_start(
                            out=xc[:, 2 * q8 : 2 * q8 + 2, :],
                            in_=xin[:, 2 * q8 : 2 * q8 + 2, :],
                        )
                else:
                    nc.sync.dma_start(out=xc[:, 0:8, :], in_=xin[:, 0:8, :])
                    nc.sync.dma_start(out=xc[:, 8:NE, :], in_=xin[:, 8:NE, :])

                # K projection: K^T [Dh, 512] for this window
                kp = ppsum.tile([128, WTOK], F32, tag="proj")
                for e in range(NE):
                    nc.tensor.matmul(
                        kp, w_sb["wk"][:, e, :], xc[:, e, :],
                        start=(e == 0), stop=(e == NE - 1),
                    )
                nc.vector.tensor_copy(kt_sb[:, w * WTOK : (w + 1) * WTOK], kp)

                # V projection: V^T [Dh, 512], rope, transpose to natural
                vp = ppsum.tile([128, WTOK], F32, tag="proj")
                for e in range(NE):
                    nc.tensor.matmul(
                        vp, w_sb["wv"][:, e, :], xc[:, e, :],
                        start=(e == 0), stop=(e == NE - 1),
                    )
                vt = work.tile([128, WTOK], BF16, tag="vt")
                csw = work.tile([64, WTOK], F32, tag="csw", name=f"csw_{w}")
                snw = work.tile([64, WTOK], F32, tag="snw", name=f"snw_{w}")
                nc.sync.dma_start(out=csw, in_=cosv[:, w * WTOK : (w + 1) * WTOK])
                nc.sync.dma_start(out=snw, in_=sinv[:, w * WTOK : (w + 1) * WTOK])
                cs = csw[:, :]
                sn = snw[:, :]
                t1 = rope.tile([64, WTOK], F32, tag="t1")
                t2 = rope.tile([64, WTOK], F32, tag="t2")
                t3 = rope.tile([64, WTOK], F32, tag="t3")
                t4 = rope.tile([64, WTOK], F32, tag="t4")
                nc.vector.tensor_mul(t1, vp[0:64, :], cs)
                nc.vector.tensor_mul(t2, vp[64:128, :], sn)
                nc.vector.tensor_sub(vt[0:64, :], t1, t2)
                nc.vector.tensor_mul(t3, vp[0:64, :], sn)
                nc.vector.tensor_mul(t4, vp[64:128, :], cs)
                nc.vector.tensor_add(vt[64:128, :], t3, t4)
                vtr = ppsum.tile([128, 4, 128], BF16, tag="proj", name=f"vtr_{w}")
                for s in range(4):
                    nc.tensor.transpose(
                        vtr[:, s, :], vt[:, s * 128 : (s + 1) * 128], ident
                    )
                nc.vector.tensor_copy(vn_sb[:, 4 * w : 4 * w + 4, 0:128], vtr)

                # Q projection for pair w: q columns at window offsets
                # {0, 256} (see module docstring for the h=1 rotation trick)
                qp = ppsum.tile([128, 256], F32, tag="proj")
                for e in range(NE):
                    rhs = xc[:, e, :].rearrange(
                        "p (a t b) -> p a t b", a=2, t=2
                    )[:, :, 0:1, :]
                    nc.tensor.matmul(
                        qp, w_sb["wq"][:, e, :], rhs,
                        start=(e == 0), stop=(e == NE - 1),
                    )
                qcs = cs.rearrange("p (a t b) -> p a t b", a=2, t=2)[:, :, 0:1, :]
                qsn = sn.rearrange("p (a t b) -> p a t b", a=2, t=2)[:, :, 0:1, :]
                q1 = rope.tile([64, 256], F32, tag="q1")
                q2 = rope.tile([64, 256], F32, tag="q2")
                q3 = rope.tile([64, 256], F32, tag="q3")
                q4 = rope.tile([64, 256], F32, tag="q4")
                qdst = qt_sb[:, 256 * w : 256 * (w + 1)]
                nc.vector.tensor_mul(q1, qp[0:64, :], qcs)
                nc.vector.tensor_mul(q2, qp[64:128, :], qsn)
                nc.vector.tensor_sub(qdst[0:64, :], q1, q2)
                nc.vector.tensor_mul(q3, qp[0:64, :], qsn)
                nc.vector.tensor_mul(q4, qp[64:128, :], qcs)
                nc.vector.tensor_add(qdst[64:128, :], q3, q4)

            # ---- attention: S^T scores, exp, P^T-stationary AV+denom ----
            for t in range(NPAIR):
                acc0 = apsum.tile([128, 132], F32, tag="acc0", name=f"acc0_{t}")
                acc1 = apsum.tile([128, 132], F32, tag="acc1", name=f"acc1_{t}")
                accs = [acc0, acc1]
                n_mm = 4 * (t + 1)
                mi = 0
                for kt in range(t + 1):
                    st = spsum.tile([128, 4, 256], F32, tag="st")
                    for i in range(4):
                        g = 4 * kt + i
                        nc.tensor.matmul(
                            st[:, i, :],
                            kt_sb[:, g * 128 : (g + 1) * 128],
                            qt_sb[:, 256 * t : 256 * (t + 1)],
                            start=True, stop=True,
                        )
                    pt = ptp.tile([128, 4, 256], BF16, tag="pt")
                    if kt == t:
                        praw = ptp.tile([128, 4, 256], BF16, tag="praw")
                        nc.scalar.activation(
                            praw, st, mybir.ActivationFunctionType.Exp,
                            bias=0.0, scale=SCALE,
                        )
                        nc.vector.tensor_mul(pt, praw, mask_sb)
                    else:
                        nc.scalar.activation(
                            pt, st, mybir.ActivationFunctionType.Exp,
                            bias=0.0, scale=SCALE,
                        )
                    for i in range(4):
                        g = 4 * kt + i
                        first = mi == 0
                        last = mi == n_mm - 1
                        for blk in range(2):
                            nc.tensor.matmul(
                                accs[blk][:, 0:129],
                                pt[:, i, blk * 128 : (blk + 1) * 128],
                                vn_sb[:, g, 0:129],
                                start=first, stop=last,
                            )
                        mi += 1

                if debug:
                    for blk in range(2):
                        accd = fin.tile([128, 129], F32, tag="accd")
                        nc.vector.tensor_copy(accd, accs[blk][:, 0:129])
                        nc.sync.dma_start(out=acc_dbg[t, blk, :, :], in_=accd)
                for blk in range(2):
                    recip = fin.tile([128, 1], F32, tag="recip")
                    nc.vector.reciprocal(recip, accs[blk][:, 128:129])
                    nc.vector.tensor_scalar_mul(
                        out_sb[:, 2 * t + blk, :], accs[blk][:, 0:128], recip
                    )
                nc.sync.dma_start(
                    out=outt[:, 2 * t : 2 * t + 2, :],
                    in_=out_sb[:, 2 * t : 2 * t + 2, :],
                )

            if debug:
                nc.sync.dma_start(out=kt_dbg[:, :], in_=kt_sb)
                nc.sync.dma_start(out=qt_dbg[:, :], in_=qt_sb)
                nc.sync.dma_start(out=vn_dbg[:, :, :], in_=vn_sb)

    nc.compile()
    return nc


def _rope_tables():
    p = np.arange(64, dtype=np.float64)
    inv = 10000.0 ** (-2.0 * p / DH)
    pos = np.arange(S, dtype=np.float64)
    theta = inv[:, None] * pos[None, :]
    return (np.cos(theta).astype(np.float32), np.sin(theta).astype(np.float32))


def _masks_rotated(h):
    """{0,1} mask for the diagonal (last) k-tile, in the core's window-local
    k order: local k position 128*s + r maps to global window offset krel
    (identity for h=0; rotated by +128 mod 512 for h=1).  q pair columns:
    block A (cols 0:128) sits at window offset 128*h, block B (cols 128:256)
    at 128*(2+h).  Valid iff krel <= q offset."""
    r = np.arange(128)[:, None]
    i = np.arange(128)[None, :]
    m = np.zeros((128, 4, 256), dtype=np.float32)
    for s in range(4):
        krel = 128 * s + r if h == 0 else (128 * s + r + 128) % 512
        m[:, s, 0:128] = krel <= (128 * h + i)
        m[:, s, 128:256] = krel <= (128 * (2 + h) + i)
    return m.astype(ml_dtypes.bfloat16)


def _xcast(a):
    if XDT == BF16:
        return np.ascontiguousarray(a.astype(ml_dtypes.bfloat16))
    return np.ascontiguousarray(a)


def kernel(x, W_Q, W_K, W_V):
    x = np.asarray(x, dtype=np.float32)
    W_Q = np.asarray(W_Q, dtype=np.float32)
    W_K = np.asarray(W_K, dtype=np.float32)
    W_V = np.asarray(W_V, dtype=np.float32)

    if "nc" not in _CACHE:
        _CACHE["nc"] = _build()
    nc = _CACHE["nc"]

    perm = np.concatenate([np.arange(0, DH, 2), np.arange(1, DH, 2)])
    cos_t, sin_t = _rope_tables()

    wq_h = _xcast(W_Q[:, perm].reshape(NE, 128, DH).transpose(1, 0, 2))
    wk_h = _xcast(W_K[:, perm].reshape(NE, 128, DH).transpose(1, 0, 2))
    wv_h = _xcast(W_V[:, perm].reshape(NE, 128, DH).transpose(1, 0, 2))

    cos_rot = np.ascontiguousarray(
        np.roll(cos_t.reshape(64, NW, WTOK), -128, axis=2).reshape(64, S)
    )
    sin_rot = np.ascontiguousarray(
        np.roll(sin_t.reshape(64, NW, WTOK), -128, axis=2).reshape(64, S)
    )
    masks = [_masks_rotated(0), _masks_rotated(1)]

    in_maps = []
    metas = []
    for b in range(B):
        xt_b = x[b].T.reshape(NE, 128, NW, WTOK)
        for h in range(2):
            if h == 0:
                xt_c = _xcast(xt_b.transpose(2, 1, 0, 3))
                cos_c, sin_c = cos_t, sin_t
            else:
                rot = np.roll(xt_b, -128, axis=3)
                xt_c = _xcast(rot.transpose(2, 1, 0, 3))
                cos_c, sin_c = cos_rot, sin_rot
            in_maps.append(
                {
                    "xt": xt_c,
                    "wq": wq_h,
                    "wk": wk_h,
                    "wv": wv_h,
                    "cosv": cos_c,
                    "sinv": sin_c,
                    "maskt": masks[h],
                }
            )
            metas.append((b, h))

    global _LAST_IN_MAPS
    _LAST_IN_MAPS = in_maps

    try:
        res = run_bass_kernel_spmd(nc, in_maps, list(range(8)))
    except Exception:
        # transient NRT device errors have been observed; retry once
        import time as _time

        _time.sleep(2.0)
        res = run_bass_kernel_spmd(nc, in_maps, list(range(8)))

    out = np.empty((B, S, DH), dtype=np.float32)
    for c, (b, h) in enumerate(metas):
        ot = res.results[c]["outt"]          # [128 q, 16 blocks, 128 dh-perm]
        for j in range(16):
            g = 2 * j + h
            out[b, g * 128 : (g + 1) * 128, :][:, perm] = ot[:, j, :]
    return out


# revision 31
# speedup vs baseline: 1.0201x; 1.0201x over previous
"""Trainium2 Bass kernel for a single causal attention head (prefill).

Problem shapes (hardcoded): x [4, 4096, 2048], W_Q/W_K/W_V [2048, 128].
reference: Q = rope(x@W_Q), K = x@W_K, V = rope(x@W_V),
           out = softmax(causal(Q K^T / sqrt(128))) @ V.

Sharding: 8 cores = (batch b, stripe h in {0,1}).  Core (b,h) computes the
output rows of batch b belonging to the interleaved 128-row blocks
g = 2j + h (j = 0..15) — balancing the causal triangle between the two
cores of a batch.  Each core computes the full K/V of its batch locally
(no collectives).

On-chip layout: everything transposed.  The host passes x^T per batch, so
projections contract over E with W e-tiles stationary and x^T moving,
producing Q^T/K^T/V^T [Dh=128 part, tok free] directly.  RoPE pairs are
made partition-contiguous by permuting W columns (even dims first) on the
host.  Scores are computed transposed (S^T[k, q]); softmax skips the max
subtraction (scores are gaussian-bounded, exp stays in fp32 range); exp
runs on ACT; P^T @ V and the softmax denominator (ones-matmul) accumulate
in a single PSUM bank per 256-row q-pair; the output is produced
transposed and rearranged on the host.

SPMD trick for the h stripe: both cores run one program that always
gathers q columns at window offsets {0, 256}.  For h=1 cores the host
rotates every 512-token window of x^T (and the rope tables) left by 128
columns, so those offsets select the h=1 blocks; K/V column order inside a
window changes with it, which only matters for the causal mask of the
diagonal k-tile — and the host builds that mask for the rotated order.
"""

import os
import sys

for _p in (
    "/root/.axon_site",
    "/root/.axon_site/_ro/trn_rl_repo",
    "/root/.axon_site/_ro/pypackages",
    "/opt/trn_rl_repo",
):
    if os.path.isdir(_p) and _p not in sys.path:
        sys.path.append(_p)

import numpy as np
import ml_dtypes

import concourse.mybir as mybir
import concourse.tile as tile
from concourse import bacc
from concourse.bass_utils import run_bass_kernel_spmd
from concourse.masks import make_identity

B, S, E, DH = 4, 4096, 2048, 128
NE = E // 128            # 16 e-tiles
NW = 8                   # windows of 512 tokens
WTOK = S // NW           # 512
NPAIR = 8                # q pairs per core (each 256 q rows)
NKSUB = S // 128         # 32 global k sub-blocks of 128
SCALE = 1.0 / float(np.sqrt(np.float64(DH)))
F32, F32R, BF16 = mybir.dt.float32, mybir.dt.float32r, mybir.dt.bfloat16
XDT = BF16  # dtype of x^T and W inputs (BF16 halves DMA; F32R for accuracy)

_CACHE = {}


def _build(debug=False):
    nc = bacc.Bacc()

    xt = nc.dram_tensor("xt", [NW, 128, NE, WTOK], XDT, kind="ExternalInput")
    wq = nc.dram_tensor("wq", [128, NE, DH], XDT, kind="ExternalInput")
    wk = nc.dram_tensor("wk", [128, NE, DH], XDT, kind="ExternalInput")
    wv = nc.dram_tensor("wv", [128, NE, DH], XDT, kind="ExternalInput")
    cosv = nc.dram_tensor("cosv", [64, S], F32, kind="ExternalInput")
    sinv = nc.dram_tensor("sinv", [64, S], F32, kind="ExternalInput")
    maskt = nc.dram_tensor("maskt", [128, 4, 256], BF16, kind="ExternalInput")
    outt = nc.dram_tensor("outt", [128, 16, 128], F32, kind="ExternalOutput")
    if debug:
        kt_dbg = nc.dram_tensor("kt_dbg", [128, S], F32R, kind="ExternalOutput")
        qt_dbg = nc.dram_tensor("qt_dbg", [128, 2048], F32R, kind="ExternalOutput")
        vn_dbg = nc.dram_tensor("vn_dbg", [128, NKSUB, 132], BF16, kind="ExternalOutput")
        acc_dbg = nc.dram_tensor("acc_dbg", [8, 2, 128, 129], F32, kind="ExternalOutput")

    with tile.TileContext(nc) as tc:
        with (
            tc.tile_pool(name="consts", bufs=1) as consts,
            tc.tile_pool(name="big", bufs=1) as big,
            tc.tile_pool(name="xs", bufs=3) as xs,
            tc.tile_pool(name="work", bufs=3) as work,
            tc.tile_pool(name="rope", bufs=2) as rope,
            tc.tile_pool(name="pt", bufs=8) as ptp,
            tc.tile_pool(name="fin", bufs=4) as fin,
            tc.tile_pool(name="ppsum", bufs=2, space="PSUM") as ppsum,
            tc.tile_pool(name="spsum", bufs=2, space="PSUM") as spsum,
            tc.tile_pool(name="apsum", bufs=1, space="PSUM") as apsum,
        ):
            w_sb = {}
            for name, dram in (("wk", wk), ("wv", wv), ("wq", wq)):
                t = consts.tile([128, NE, DH], XDT, tag=name)
                if name == "wk":
                    nc.scalar.dma_start(out=t[:, 0:2, :], in_=dram[:, 0:2, :])
                    nc.scalar.dma_start(out=t[:, 2:NE, :], in_=dram[:, 2:NE, :])
                else:
                    nc.scalar.dma_start(out=t, in_=dram[:, :, :])
                w_sb[name] = t
            mask_sb = consts.tile([128, 4, 256], BF16, tag="maskt")
            nc.scalar.dma_start(out=mask_sb, in_=maskt[:, :, :])

            ident = consts.tile([128, 128], BF16, tag="ident")
            make_identity(nc, ident)

            kt_sb = big.tile([128, S], F32R, tag="kt")
            qt_sb = big.tile([128, 2048], F32R, tag="qt")
            vn_sb = big.tile([128, NKSUB, 132], BF16, tag="vn")
            nc.vector.memset(vn_sb[:, :, 128:129], 1.0)
            out_sb = big.tile([128, 16, 128], F32, tag="outt")

            # ---- projections + rope, one 512-token window at a time ----
            for w in range(NW):
                xc = xs.tile([128, NE, WTOK], XDT, tag="xc")
                xin = xt[w, :, :, :]
                if w == 0:
                    for q8 in range(8):
                        nc.sync.dma_start(
                            out=xc[:, 2 * q8 : 2 * q8 + 2, :],
                            in_=xin[:, 2 * q8 : 2 * q8 + 2, :],
                        )
                else:
                    nc.sync.dma_start(out=xc[:, 0:8, :], in_=xin[:, 0:8, :])
                    nc.sync.dma_start(out=xc[:, 8:NE, :], in_=xin[:, 8:NE, :])

                # K projection: K^T [Dh, 512] for this window
                kp = ppsum.tile([128, WTOK], F32, tag="proj")
                for e in range(NE):
                    nc.tensor.matmul(
                        kp, w_sb["wk"][:, e, :], xc[:, e, :],
                        start=(e == 0), stop=(e == NE - 1),
                    )
                nc.vector.tensor_copy(kt_sb[:, w * WTOK : (w + 1) * WTOK], kp)

                # V projection: V^T [Dh, 512], rope, transpose to natural
                vp = ppsum.tile([128, WTOK], F32, tag="proj")
                for e in range(NE):
                    nc.tensor.matmul(
                        vp, w_sb["wv"][:, e, :], xc[:, e, :],
                        start=(e == 0), stop=(e == NE - 1),
                    )
                vt = work.tile([128, WTOK], BF16, tag="vt")
                csw = work.tile([64, WTOK], F32, tag="csw", name=f"csw_{w}")
                snw = work.tile([64, WTOK], F32, tag="snw", name=f"snw_{w}")
                nc.sync.dma_start(out=csw, in_=cosv[:, w * WTOK : (w + 1) * WTOK])
                nc.sync.dma_start(out=snw, in_=sinv[:, w * WTOK : (w + 1) * WTOK])
                cs = csw[:, :]
                sn = snw[:, :]
                t1 = rope.tile([64, WTOK], F32, tag="t1")
                t2 = rope.tile([64, WTOK], F32, tag="t2")
                t3 = rope.tile([64, WTOK], F32, tag="t3")
                t4 = rope.tile([64, WTOK], F32, tag="t4")
                nc.vector.tensor_mul(t1, vp[0:64, :], cs)
                nc.vector.tensor_mul(t2, vp[64:128, :], sn)
                nc.vector.tensor_sub(vt[0:64, :], t1, t2)
                nc.vector.tensor_mul(t3, vp[0:64, :], sn)
                nc.vector.tensor_mul(t4, vp[64:128, :], cs)
                nc.vector.tensor_add(vt[64:128, :], t3, t4)
                vtr = ppsum.tile([128, 4, 128], BF16, tag="proj", name=f"vtr_{w}")
                for s in range(4):
                    nc.tensor.transpose(
                        vtr[:, s, :], vt[:, s * 128 : (s + 1) * 128], ident
                    )
                nc.vector.tensor_copy(vn_sb[:, 4 * w : 4 * w + 4, 0:128], vtr)

                # Q projection for pair w: q columns at window offsets
                # {0, 256} (see module docstring for the h=1 rotation trick)
                qp = ppsum.tile([128, 256], F32, tag="proj")
                for e in range(NE):
                    rhs = xc[:, e, :].rearrange(
                        "p (a t b) -> p a t b", a=2, t=2
                    )[:, :, 0:1, :]
                    nc.tensor.matmul(
                        qp, w_sb["wq"][:, e, :], rhs,
                        start=(e == 0), stop=(e == NE - 1),
                    )
                qcs = cs.rearrange("p (a t b) -> p a t b", a=2, t=2)[:, :, 0:1, :]
                qsn = sn.rearrange("p (a t b) -> p a t b", a=2, t=2)[:, :, 0:1, :]
                q1 = rope.tile([64, 256], F32, tag="q1")
                q2 = rope.tile([64, 256], F32, tag="q2")
                q3 = rope.tile([64, 256], F32, tag="q3")
                q4 = rope.tile([64, 256], F32, tag="q4")
                qdst = qt_sb[:, 256 * w : 256 * (w + 1)]
                nc.vector.tensor_mul(q1, qp[0:64, :], qcs)
                nc.vector.tensor_mul(q2, qp[64:128, :], qsn)
                nc.vector.tensor_sub(qdst[0:64, :], q1, q2)
                nc.vector.tensor_mul(q3, qp[0:64, :], qsn)
                nc.vector.tensor_mul(q4, qp[64:128, :], qcs)
                nc.vector.tensor_add(qdst[64:128, :], q3, q4)

            # ---- attention: S^T scores, exp, P^T-stationary AV+denom ----
            for t in range(NPAIR):
                acc0 = apsum.tile([128, 132], F32, tag="acc0", name=f"acc0_{t}")
                acc1 = apsum.tile([128, 132], F32, tag="acc1", name=f"acc1_{t}")
                accs = [acc0, acc1]
                n_mm = 4 * (t + 1)
                mi = 0
                for kt in range(t + 1):
                    st = spsum.tile([128, 4, 256], F32, tag="st")
                    for i in range(4):
                        g = 4 * kt + i
                        nc.tensor.matmul(
                            st[:, i, :],
                            kt_sb[:, g * 128 : (g + 1) * 128],
                            qt_sb[:, 256 * t : 256 * (t + 1)],
                            start=True, stop=True,
                        )
                    pt = ptp.tile([128, 4, 256], BF16, tag="pt")
                    if kt == t:
                        praw = ptp.tile([128, 4, 256], BF16, tag="praw")
                        nc.scalar.activation(
                            praw, st, mybir.ActivationFunctionType.Exp,
                            bias=0.0, scale=SCALE,
                        )
                        nc.vector.tensor_mul(pt, praw, mask_sb)
                    else:
                        nc.scalar.activation(
                            pt, st, mybir.ActivationFunctionType.Exp,
                            bias=0.0, scale=SCALE,
                        )
                    for i in range(4):
                        g = 4 * kt + i
                        first = mi == 0
                        last = mi == n_mm - 1
                        for blk in range(2):
                            nc.tensor.matmul(
                                accs[blk][:, 0:129],
                                pt[:, i, blk * 128 : (blk + 1) * 128],
                                vn_sb[:, g, 0:129],
                                start=first, stop=last,
                            )
                        mi += 1

                if debug:
                    for blk in range(2):
                        accd = fin.tile([128, 129], F32, tag="accd")
                        nc.vector.tensor_copy(accd, accs[blk][:, 0:129])
                        nc.sync.dma_start(out=acc_dbg[t, blk, :, :], in_=accd)
                for blk in range(2):
                    recip = fin.tile([128, 1], F32, tag="recip")
                    nc.vector.reciprocal(recip, accs[blk][:, 128:129])
                    nc.vector.tensor_scalar_mul(
                        out_sb[:, 2 * t + blk, :], accs[blk][:, 0:128], recip
                    )
                nc.sync.dma_start(
                    out=outt[:, 2 * t : 2 * t + 2, :],
                    in_=out_sb[:, 2 * t : 2 * t + 2, :],
                )

            if debug:
                nc.sync.dma_start(out=kt_dbg[:, :], in_=kt_sb)
                nc.sync.dma_start(out=qt_dbg[:, :], in_=qt_sb)
                nc.sync.dma_start(out=vn_dbg[:, :, :], in_=vn_sb)

    nc.compile()
    return nc


def _rope_tables():
    p = np.arange(64, dtype=np.float64)
    inv = 10000.0 ** (-2.0 * p / DH)
    pos = np.arange(S, dtype=np.float64)
    theta = inv[:, None] * pos[None, :]
    return (np.cos(theta).astype(np.float32), np.sin(theta).astype(np.float32))


def _masks_rotated(h):
    """{0,1} mask for the diagonal (last) k-tile, in the core's window-local
    k order: local k position 128*s + r maps to global window offset krel
    (identity for h=0; rotated by +128 mod 512 for h=1).  q pair columns:
    block A (cols 0:128) sits at window offset 128*h, block B (cols 128:256)
    at 128*(2+h).  Valid iff krel <= q offset."""
    r = np.arange(128)[:, None]
    i = np.arange(128)[None, :]
    m = np.zeros((128, 4, 256), dtype=np.float32)
    for s in range(4):
        krel = 128 * s + r if h == 0 else (128 * s + r + 128) % 512
        m[:, s, 0:128] = krel <= (128 * h + i)
        m[:, s, 128:256] = krel <= (128 * (2 + h) + i)
    return m.astype(ml_dtypes.bfloat16)


def _xcast(a):
    if XDT == BF16:
        return np.ascontiguousarray(a.astype(ml_dtypes.bfloat16))
    return np.ascontiguousarray(a)


def kernel(x, W_Q, W_K, W_V):
    x = np.asarray(x, dtype=np.float32)
    W_Q = np.asarray(W_Q, dtype=np.float32)
    W_K = np.asarray(W_K, dtype=np.float32)
    W_V = np.asarray(W_V, dtype=np.float32)

    if "nc" not in _CACHE:
        _CACHE["nc"] = _build()
    nc = _CACHE["nc"]

    perm = np.concatenate([np.arange(0, DH, 2), np.arange(1, DH, 2)])
    cos_t, sin_t = _rope_tables()

    wq_h = _xcast(W_Q[:, perm].reshape(NE, 128, DH).transpose(1, 0, 2))
    wk_h = _xcast(W_K[:, perm].reshape(NE, 128, DH).transpose(1, 0, 2))
    wv_h = _xcast(W_V[:, perm].reshape(NE, 128, DH).transpose(1, 0, 2))

    cos_rot = np.ascontiguousarray(
        np.roll(cos_t.reshape(64, NW, WTOK), -128, axis=2).reshape(64, S)
    )
    sin_rot = np.ascontiguousarray(
        np.roll(sin_t.reshape(64, NW, WTOK), -128, axis=2).reshape(64, S)
    )
    masks = [_masks_rotated(0), _masks_rotated(1)]

    in_maps = []
    metas = []
    for b in range(B):
        xt_b = x[b].T.reshape(NE, 128, NW, WTOK)
        for h in range(2):
            if h == 0:
                xt_c = _xcast(xt_b.transpose(2, 1, 0, 3))
                cos_c, sin_c = cos_t, sin_t
            else:
                rot = np.roll(xt_b, -128, axis=3)
                xt_c = _xcast(rot.transpose(2, 1, 0, 3))
                cos_c, sin_c = cos_rot, sin_rot
            in_maps.append(
                {
                    "xt": xt_c,
                    "wq": wq_h,
                    "wk": wk_h,
                    "wv": wv_h,
                    "cosv": cos_c,
                    "sinv": sin_c,
                    "maskt": masks[h],
                }
            )
            metas.append((b, h))

    global _LAST_IN_MAPS
    _LAST_IN_MAPS = in_maps

    try:
        res = run_bass_kernel_spmd(nc, in_maps, list(range(8)))
    except Exception:
        # transient NRT device errors have been observed; retry once
        import time as _time

        _time.sleep(2.0)
        res = run_bass_kernel_spmd(nc, in_maps, list(range(8)))

    out = np.empty((B, S, DH), dtype=np.float32)
    for c, (b, h) in enumerate(metas):
        ot = res.results[c]["outt"]          # [128 q, 16 blocks, 128 dh-perm]
        for j in range(16):
            g = 2 * j + h
            out[b, g * 128 : (g + 1) * 128, :][:, perm] = ot[:, j, :]
    return out
